# revision 12
# baseline (speedup 1.0000x reference)
"""CrossAttention3D kernel for Trainium2 (Bass/Tile), SPMD over 8 NeuronCores.

Problem (full shapes): q_inputs [4,4096,128], kv_inputs [4,4096,128],
Wq/Wk/Wv [128,128], bq/bk/bv [128].
    q = q_in @ Wq + bq ; k = kv_in @ Wk + bk ; v = kv_in @ Wv + bv
    out = softmax(q k^T / sqrt(128)) @ v

Sharding: data-parallel over batch (4) x query-sequence halves (2) = 8 shards.
Each core: xqT [128, 2048] (transposed query slice), xkvT [128, 4096]
(transposed kv for its batch) -- the host pre-transposes inputs (pure layout
marshaling) so C lands on partitions with contiguous DMA lines, and
un-transposes the [F, NQ] output.  No on-device input/output transposes.

v3 design:
  - Weight folding: scores == Q2 @ Xkv^T up to per-row constants that cancel
    in softmax, where Q2 = Xq (Wq Wk^T) + Wk^T bq.  No k-projection; the
    bf16-cast kvT is used directly as the score weights.
  - vt tiles [m,f] = kvT_block^T @ Wv (PV weights) computed by matmul, no
    re-transpose.  bv enters via a rank-1 PSUM-accumulated matmul
    oT += bv (x) d at the end (out = (sum E v + bv*d)/d = out_true).
  - bf16 attention core: same 1 cyc/col matmul rate as f32r, but halves
    eviction bytes and unlocks DVE 2-byte perf modes for denominator adds.
  - Denominator: exp tiles accumulated into two bf16 SBUF accs (even/odd kv
    tile; split DVE/GpSimd) via scalar_tensor_tensor (4x_2p on DVE), folded
    over partitions by ones-weight matmuls, broadcast, reciprocal, one fused
    multiply on eviction.
  - exp split: most tiles on ACT (Exp, scale folded); a subset on DVE via
    the Schraudolph bit trick: bf16bits(exp(x*SCALE)) ~= int16(x*C1 + C2),
    one tensor_scalar into int16, bitcast to bf16 (end-to-end adds ~3e-3).
  - GPSIMD never touches PSUM (hardware restriction): it gets SBUF-only work
    (input casts, some denominator adds, partition broadcasts).
  - PE p-state: TRN2 PE runs ~1.2GHz until ~3us of gapless execution, then
    2.4GHz; emission keeps the PE dense (preamble interleaved with chunk-0).
"""

import math
from contextlib import ExitStack

import numpy as np

P = 128
B_FULL, NQ_FULL, NKV, C, F = 4, 4096, 4096, 128, 128
N_CORES = 8
NQ = B_FULL * NQ_FULL // N_CORES  # 2048 queries per core
SCALE = 1.0 / math.sqrt(F)

NKV_T = NKV // P  # 32 kv tiles
NCHUNK = 1024
NCH = NQ // NCHUNK  # 2 chunks
MM = 512  # max moving free dim
NSL_Q = NQ // MM  # 4 q column slices
NSL_K = NKV // MM  # 8 kv column slices

# Schraudolph exp constants (bf16 bit pattern via int16):
#   bf16_bits(exp(s*SCALE)) ~= round(s * SCALE*128/ln2 + 127*128 - 7.25)
EXP_C1 = SCALE * 128.0 / math.log(2.0)
EXP_C2 = 127.0 * 128.0 - 7.25

# per-chunk engine assignment patterns (by kv tile index mi)
SCHRAUD_DVE = frozenset(mi for mi in range(NKV_T) if mi % 4 == 2)  # 8/chunk
GPS_ADD = frozenset({5, 13, 21, 29})  # denominator adds on GpSimd, 4/chunk

_CACHE = {}


def _build_nc():
    import concourse.bacc as bacc
    import concourse.tile as tile
    from concourse import mybir
    from concourse.masks import make_identity

    FP32 = mybir.dt.float32
    BF16 = mybir.dt.bfloat16
    I16 = mybir.dt.int16
    ADD = mybir.AluOpType.add
    MULT = mybir.AluOpType.mult

    nc = bacc.Bacc("TRN2", target_bir_lowering=False, debug=False)

    xqT = nc.dram_tensor("xqT", [C, NQ], FP32, kind="ExternalInput")
    xkvT = nc.dram_tensor("xkvT", [C, NKV], FP32, kind="ExternalInput")
    wq = nc.dram_tensor("wq", [C, F], FP32, kind="ExternalInput")
    wk = nc.dram_tensor("wk", [C, F], FP32, kind="ExternalInput")
    wv = nc.dram_tensor("wv", [C, F], FP32, kind="ExternalInput")
    bq = nc.dram_tensor("bq", [F, 1], FP32, kind="ExternalInput")
    bv = nc.dram_tensor("bv", [F, 1], FP32, kind="ExternalInput")
    outT = nc.dram_tensor("outT", [F, NQ], FP32, kind="ExternalOutput")

    with tile.TileContext(nc) as tc, ExitStack() as ctx:
        const = ctx.enter_context(tc.tile_pool(name="const", bufs=1))
        identity = const.tile([P, P], FP32)
        make_identity(nc, identity)

        pwork = ctx.enter_context(tc.tile_pool(name="pwork", bufs=2, space="PSUM"))
        spsum = ctx.enter_context(tc.tile_pool(name="spsum", bufs=2, space="PSUM"))
        opsum = ctx.enter_context(tc.tile_pool(name="opsum", bufs=1, space="PSUM"))
        epool = ctx.enter_context(tc.tile_pool(name="epool", bufs=6))
        apool = ctx.enter_context(tc.tile_pool(name="apool", bufs=4))
        npool = ctx.enter_context(tc.tile_pool(name="npool", bufs=2))
        onpool = ctx.enter_context(tc.tile_pool(name="onpool", bufs=2))

        # ---- weight DMAs first (A-setup is the first PE work) ----
        wq_raw = const.tile([C, F], FP32, name="wq_raw")
        nc.sync.dma_start(wq_raw, wq[:])
        wk_raw = const.tile([C, F], FP32, name="wk_raw")
        nc.sync.dma_start(wk_raw, wk[:])
        wv_raw = const.tile([C, F], FP32, name="wv_raw")
        nc.sync.dma_start(wv_raw, wv[:])
        bq_s = const.tile([F, 1], FP32)
        nc.sync.dma_start(bq_s, bq[:])
        bv_s = const.tile([F, 1], FP32)
        nc.sync.dma_start(bv_s, bv[:])

        # ---- input staging: sliced DMAs so casts can start early ----
        qstage = const.tile([P, NQ], FP32, name="qstage")
        for j in range(NSL_Q):
            nc.sync.dma_start(
                qstage[:, j * MM : (j + 1) * MM], xqT[:, j * MM : (j + 1) * MM]
            )
        kstage = const.tile([P, NKV], FP32, name="kstage")
        for j in range(NSL_K):
            nc.sync.dma_start(
                kstage[:, j * MM : (j + 1) * MM], xkvT[:, j * MM : (j + 1) * MM]
            )

        ones_b = const.tile([P, 1], BF16)
        nc.vector.memset(ones_b, 1.0)
        wv_b = const.tile([C, F], BF16)
        nc.vector.tensor_copy(wv_b, wv_raw)

        # ---- A = Wq Wk^T (bf16), cvec = Wk^T bq ----
        wqT_p = pwork.tile([F, C], FP32, tag="work", name="wqT_p")
        nc.tensor.transpose(wqT_p, wq_raw, identity)
        wqT_s = const.tile([F, C], FP32)
        nc.scalar.copy(wqT_s, wqT_p)
        wkT_p = pwork.tile([F, C], FP32, tag="work", name="wkT_p")
        nc.tensor.transpose(wkT_p, wk_raw, identity)
        wkT_s = const.tile([F, C], FP32)
        nc.scalar.copy(wkT_s, wkT_p)

        a_p = pwork.tile([C, C], FP32, tag="work", name="a_p")
        nc.tensor.matmul(a_p, wqT_s, wkT_s, start=True, stop=True)
        a_s = const.tile([C, C], BF16)
        nc.vector.tensor_copy(a_s, a_p)

        cv_p = pwork.tile([C, 1], FP32, tag="work", name="cv_p")
        nc.tensor.matmul(cv_p, wkT_s, bq_s, start=True, stop=True)
        cvec = const.tile([C, 1], FP32)
        nc.vector.tensor_copy(cvec, cv_p)

        # ---- persistent SBUF tensors ----
        kvT = const.tile([P, NKV], BF16)  # [c, m] bf16 keys
        qTin = const.tile([P, NQ], BF16)  # [c, n] bf16 queries
        q2T = const.tile([P, NQ], BF16)  # [c2, n] = (Xq A + cvec)^T
        vt = const.tile([P, NKV_T, F], BF16)  # [m%128, m//128, f] PV weights

        def load_q_slice(j):
            """Cast one 512-col q slice to bf16, project through A (+cvec)."""
            sl = slice(j * MM, (j + 1) * MM)
            nc.vector.tensor_copy(qTin[:, sl], qstage[:, sl])
            q2p = pwork.tile([P, MM], FP32, tag="work", name=f"q2p_{j}")
            nc.tensor.matmul(q2p, a_s, qTin[:, sl], start=True, stop=True)
            nc.vector.tensor_scalar_add(q2T[:, sl], q2p, cvec)

        def load_kv_slice(j, cast_eng):
            """Cast one 512-col kv slice to bf16; build its 4 vt tiles."""
            sl = slice(j * MM, (j + 1) * MM)
            cast_eng.tensor_copy(kvT[:, sl], kstage[:, sl])
            pv = pwork.tile([P, MM], FP32, tag="work", name=f"pv_{j}")
            for t in range(MM // P):
                i = j * (MM // P) + t
                nc.tensor.matmul(
                    pv[:, t * P : (t + 1) * P],
                    kvT[:, i * P : (i + 1) * P],
                    wv_b,
                    start=True,
                    stop=True,
                )
            nc.scalar.copy(vt[:, j * (MM // P) : (j + 1) * (MM // P), :], pv)

        # ---- attention chunk emitter (lag-1 PV + bf16 denominator accs) ----
        chunk_state = {}

        def attn_start(nch):
            oT = opsum.tile([P, NCHUNK], FP32, tag="oT", name=f"oT_{nch}")
            acc_e = apool.tile([P, NCHUNK], BF16, tag="acc", name=f"acce_{nch}")
            acc_o = apool.tile([P, NCHUNK], BF16, tag="acc", name=f"acco_{nch}")
            chunk_state[nch] = dict(oT=oT, acc=(acc_e, acc_o), prev=None)

        def emit_pv(nch, e, mi):
            st = chunk_state[nch]
            for h in range(NCHUNK // MM):
                nc.tensor.matmul(
                    st["oT"][:, h * MM : (h + 1) * MM],
                    vt[:, mi, :],
                    e[:, h * MM : (h + 1) * MM],
                    start=(mi == 0),
                    stop=(mi == NKV_T - 1),
                )
            acc = st["acc"][mi % 2]
            if mi < 2:
                nc.vector.tensor_copy(acc, e)
            elif mi in GPS_ADD:
                nc.gpsimd.tensor_tensor(acc, acc, e, ADD)
            else:
                nc.vector.scalar_tensor_tensor(acc, e, 1.0, acc, MULT, ADD)

        def attn_mi(nch, mi):
            st = chunk_state[nch]
            nq0 = nch * NCHUNK
            sp = spsum.tile([P, NCHUNK], FP32, tag="sp", name=f"sp_{nch}_{mi}")
            for h in range(NCHUNK // MM):
                nc.tensor.matmul(
                    sp[:, h * MM : (h + 1) * MM],
                    kvT[:, mi * P : (mi + 1) * P],
                    q2T[:, nq0 + h * MM : nq0 + (h + 1) * MM],
                    start=True,
                    stop=True,
                )
            if mi in SCHRAUD_DVE:
                ei = epool.tile([P, NCHUNK], I16, tag="e", name=f"ei_{nch}_{mi}")
                nc.vector.tensor_scalar(ei, sp, EXP_C1, EXP_C2, MULT, ADD)
                e = ei.bitcast(BF16)
            else:
                e = epool.tile([P, NCHUNK], BF16, tag="e", name=f"e_{nch}_{mi}")
                nc.scalar.activation(
                    e, sp, mybir.ActivationFunctionType.Exp, scale=SCALE
                )
            if st["prev"] is not None:
                emit_pv(nch, *st["prev"])
            st["prev"] = (e, mi)

        def attn_finish(nch):
            st = chunk_state[nch]
            emit_pv(nch, *st["prev"])
            acc_e, acc_o = st["acc"]
            nc.vector.scalar_tensor_tensor(acc_e, acc_o, 1.0, acc_e, MULT, ADD)
            nq0 = nch * NCHUNK
            for h in range(NCHUNK // MM):
                hs = slice(h * MM, (h + 1) * MM)
                dn = pwork.tile([1, MM], FP32, tag="work", name=f"dn_{nch}_{h}")
                nc.tensor.matmul(dn, ones_b, acc_e[:, hs], start=True, stop=True)
                dnsb = npool.tile([1, MM], FP32, tag="dnsb", name=f"dnsb_{nch}_{h}")
                nc.vector.tensor_copy(dnsb, dn)
                rb = npool.tile([P, MM], FP32, tag="rb", name=f"rb_{nch}_{h}")
                nc.gpsimd.partition_broadcast(rb, dnsb)
                rc = npool.tile([P, MM], FP32, tag="rc", name=f"rc_{nch}_{h}")
                nc.vector.reciprocal_approx_fast(rc, rb)
                on = onpool.tile([P, MM], FP32, tag="on", name=f"on_{nch}_{h}")
                # on = (rb * bv + oT) * rc  ==  oT/d + bv
                nc.vector.scalar_tensor_tensor(
                    on, rb, bv_s, st["oT"][:, hs], MULT, ADD
                )
                nc.vector.tensor_tensor(on, on, rc, MULT)
                nc.sync.dma_start(outT[:, nq0 + h * MM : nq0 + (h + 1) * MM], on)

        # ---- preamble + interleaved chunk-0 attention ----
        for j in range(2):  # q2T for chunk 0
            load_q_slice(j)

        attn_start(0)
        cast_cycle = [nc.gpsimd, nc.gpsimd, nc.vector, nc.gpsimd]
        for g in range(NSL_K):
            load_kv_slice(g, cast_cycle[g % 4])
            if g < 2:  # finish the q side for chunk 1
                load_q_slice(g + 2)
            for t in range(MM // P):
                attn_mi(0, g * (MM // P) + t)
        attn_finish(0)

        for nch in range(1, NCH):
            attn_start(nch)
            for mi in range(NKV_T):
                attn_mi(nch, mi)
            attn_finish(nch)

    nc.compile()
    return nc


def _get_nc():
    if "nc" not in _CACHE:
        _CACHE["nc"] = _build_nc()
    return _CACHE["nc"]


def run(inputs, trace=False, **kwargs):
    """Run on 8 cores; returns (full_output [4,4096,128], BassKernelResults)."""
    from concourse.bass_utils import run_bass_kernel_spmd

    q_in = np.asarray(inputs["q_inputs"], dtype=np.float32)
    kv_in = np.asarray(inputs["kv_inputs"], dtype=np.float32)
    wq = np.ascontiguousarray(np.asarray(inputs["Wq"], dtype=np.float32))
    wk = np.ascontiguousarray(np.asarray(inputs["Wk"], dtype=np.float32))
    wv = np.ascontiguousarray(np.asarray(inputs["Wv"], dtype=np.float32))
    bq = np.ascontiguousarray(np.asarray(inputs["bq"], dtype=np.float32).reshape(F, 1))
    bv_col = np.ascontiguousarray(
        np.asarray(inputs["bv"], dtype=np.float32).reshape(F, 1)
    )

    halves = NQ_FULL // NQ  # 2
    in_maps = []
    for core in range(N_CORES):
        b, h = core // halves, core % halves
        in_maps.append(
            {
                "xqT": np.ascontiguousarray(q_in[b, h * NQ : (h + 1) * NQ].T),
                "xkvT": np.ascontiguousarray(kv_in[b].T),
                "wq": wq,
                "wk": wk,
                "wv": wv,
                "bq": bq,
                "bv": bv_col,
            }
        )

    nc = _get_nc()
    res = run_bass_kernel_spmd(
        nc, in_maps, core_ids=list(range(N_CORES)), trace=trace, **kwargs
    )

    full = np.empty((B_FULL, NQ_FULL, F), dtype=np.float32)
    for core in range(N_CORES):
        b, h = core // halves, core % halves
        full[b, h * NQ : (h + 1) * NQ] = res.results[core]["outT"].T
    return full, res


def kernel(**inputs):
    full, _ = run(inputs, trace=False)
    return full
